# revision 29
# baseline (speedup 1.0000x reference)
"""Causal self-attention (B=4, T=2048, C=1024, H=16, D=64) on 8 TRN2 NeuronCores.

Sharding: 8 cores = 4 batches x 2 head-groups (8 heads each). Each core:
  - QKV projection for its (batch, head-group) column slice of w_attn,
    producing qT/kT in [d, t] layout (transposed dataflow) and v in [t, d].
  - Causal attention in scoresT layout (scores^T[k, q] comes straight out of
    the PE; softmax denominators via an appended ones-column on V; no PE
    transposes anywhere).
  - Row-sharded output projection -> per-core partial [T, C].
Host sums the two partials per batch and adds b_proj.

Matmul operands are bf16 (1 cycle/row on the PE; fp32r streams at 2) with all
accumulation in fp32 PSUM. b_attn is folded in on-device via K=1 bias
matmuls; b_proj is added on the host during the unshard reduction.
"""

import sys
import types

import numpy as np

B, T, C, H, D = 4, 2048, 1024, 16, 64
HG = 8            # heads per core
CG = HG * D       # 512 channels per group
NCORES = 8
TB = T // 128     # 16 t-blocks
QCH = T // 512    # 4 q-chunks of 512


def _register_ntff_hook():
    """Register the axon NTFF profile hook if the image's antenv lacks it."""
    try:
        import antenv
        if getattr(antenv, "axon_hooks", None) is not None:
            return
        from trn_agent_boot.trn_boot import _ntff_profile_via_ctypes
        hook = _ntff_profile_via_ctypes("/opt/axon/libaxon_pjrt.so")
        mod = types.ModuleType("antenv.axon_hooks")
        mod._hook = hook
        mod.get_axon_ntff_profile_hook = lambda: mod._hook
        mod.set_axon_ntff_profile_hook = lambda h: setattr(mod, "_hook", h)
        sys.modules["antenv.axon_hooks"] = mod
        antenv.axon_hooks = mod
    except Exception:
        pass


_NC_CACHE = {}


def _build():
    import concourse.bacc as bacc
    import concourse.mybir as mybir
    import concourse.tile as tile
    from concourse.masks import make_upper_triangular
    from contextlib import ExitStack

    F32 = mybir.dt.float32
    F32R = mybir.dt.float32r
    BF16 = mybir.dt.bfloat16
    MUL = mybir.AluOpType.mult
    EXP = mybir.ActivationFunctionType.Exp

    nc = bacc.Bacc(None, target_bir_lowering=False, debug=False)
    xT_d = nc.dram_tensor("xT", [C, T], BF16, kind="ExternalInput")
    wqk_d = nc.dram_tensor("wqk", [C, 2 * CG], BF16, kind="ExternalInput")
    wv_d = nc.dram_tensor("wv", [C, CG], BF16, kind="ExternalInput")
    wp_d = nc.dram_tensor("wp", [CG, C], BF16, kind="ExternalInput")
    bqk_d = nc.dram_tensor("bqk", [1, 2 * CG], BF16, kind="ExternalInput")
    bv_d = nc.dram_tensor("bv", [1, CG], BF16, kind="ExternalInput")
    out_d = nc.dram_tensor("out", [T, C], F32, kind="ExternalOutput")

    CT = C // 128  # 8 c-tiles of the contraction dim

    with tile.TileContext(nc) as tc, ExitStack() as ctx:
        pers = ctx.enter_context(tc.tile_pool(name="pers", bufs=1))

        # Persistent tensors.
        # Per-head qT/kT tiles in [d, t] layout. Head h's 64 d-rows live at
        # partitions (h%2)*64..(h%2)*64+64 (matching the projection PSUM
        # layout); the other 64 partitions are zeroed so QK matmuls contract
        # over a full K=128 (keeps the PE HAM clock warm; zeros add nothing).
        qTt = [pers.tile([128, T], BF16, name=f"qTt{h}") for h in range(HG)]
        kTt = [pers.tile([128, T], BF16, name=f"kTt{h}") for h in range(HG)]
        for h in range(HG):
            zs = slice(64, 128) if h % 2 == 0 else slice(0, 64)
            nc.gpsimd.memset(qTt[h][zs, :], 0.0)
            nc.gpsimd.memset(kTt[h][zs, :], 0.0)
        # v_aug[p, j, h, 0:64] = v[t=j*128+p, h*64+d]; [..., 64] = 1.0
        v_aug = pers.tile([128, TB, HG, 65], BF16, name="v_aug")
        utri = pers.tile([128, 128], BF16, name="utri")
        ones_col = pers.tile([1, 64], F32R, name="ones_col")
        ones_q = pers.tile([1, 512], BF16, name="ones_q")
        bqk_sb = pers.tile([1, 2 * CG], BF16, name="bqk_sb")
        bv_sb = pers.tile([1, CG], BF16, name="bv_sb")

        # f32r/bf16 constants staged via f32 memset + rounding copies.
        stage = pers.tile([128, 512], F32, name="stage")
        make_upper_triangular(nc, utri[:, :], val=1.0, diag=True)
        nc.vector.memset(stage[:], 1.0)
        nc.vector.tensor_copy(ones_col[:], stage[0:1, 0:64])
        nc.vector.tensor_copy(ones_q[:], stage[0:1, :])
        nc.vector.tensor_copy(
            v_aug[:, :, :, 64:65],
            stage[:, 0:128].rearrange("p (j h) -> p j h", j=TB))
        nc.sync.dma_start(bqk_sb[:], bqk_d.ap()[:])
        nc.sync.dma_start(bv_sb[:], bv_d.ap()[:])

        # Weights for the output projection, DMA'd early (space is free).
        wp_pool = ctx.enter_context(tc.tile_pool(name="wp_pool", bufs=1))
        wp_sb = [wp_pool.tile([128, C], BF16, name=f"wp{i}") for i in range(4)]
        for i in range(4):
            nc.sync.dma_start(wp_sb[i][:], wp_d.ap()[i * 128:(i + 1) * 128, :])

        # ---------------- Phase 1: QKV projection ----------------
        with tc.tile_pool(name="wqk_pool", bufs=1) as wqk_pool, \
             tc.tile_pool(name="wv_pool", bufs=1) as wv_pool, \
             tc.tile_pool(name="xq_pool", bufs=2) as xq_pool, \
             tc.tile_pool(name="p1ps", bufs=2, space="PSUM") as p1ps:
            wqk_sb = [wqk_pool.tile([128, 2 * CG], BF16, name=f"wqk{c}")
                      for c in range(CT)]
            wv_sb = [wv_pool.tile([128, CG], BF16, name=f"wv{c}")
                     for c in range(CT)]
            for c in range(CT):
                nc.sync.dma_start(wqk_sb[c][:], wqk_d.ap()[c * 128:(c + 1) * 128, :])
                nc.sync.dma_start(wv_sb[c][:], wv_d.ap()[c * 128:(c + 1) * 128, :])

            for q in range(QCH):  # t-quarters of 512
                xq = []
                for c in range(CT):
                    xt = xq_pool.tile([128, 512], BF16, name=f"xq{c}", tag=f"xq{c}")
                    nc.sync.dma_start(
                        xt[:], xT_d.ap()[c * 128:(c + 1) * 128, q * 512:(q + 1) * 512])
                    xq.append(xt)

                # V projection: v[t, c'] for the 4 t-blocks of this quarter.
                for tb in range(4):
                    pv = p1ps.tile([128, CG], F32, tag="pv")
                    for c in range(CT):
                        nc.tensor.matmul(
                            pv[:], xq[c][:, tb * 128:(tb + 1) * 128], wv_sb[c][:],
                            start=(c == 0), stop=False)
                    nc.tensor.matmul(
                        pv[:], ones_q[:, tb * 128:(tb + 1) * 128], bv_sb[:],
                        start=False, stop=True)
                    j = q * 4 + tb
                    nc.vector.tensor_copy(
                        v_aug[:, j, :, 0:64],
                        pv[:].rearrange("p (h d) -> p h d", h=HG))

                # Q/K projection (transposed). M-block m covers heads
                # (2(m%4), 2(m%4)+1) of Q (m<4) or K (m>=4).
                for m in range(8):
                    pqk = p1ps.tile([128, 512], F32, tag="pqk")
                    for c in range(CT):
                        nc.tensor.matmul(
                            pqk[:], wqk_sb[c][:, m * 128:(m + 1) * 128], xq[c][:],
                            start=(c == 0), stop=False)
                    nc.tensor.matmul(
                        pqk[:], bqk_sb[:, m * 128:(m + 1) * 128], ones_q[:],
                        start=False, stop=True)
                    dst = qTt if m < 4 else kTt
                    h0 = 2 * (m % 4)
                    sl = slice(q * 512, (q + 1) * 512)
                    nc.vector.tensor_copy(dst[h0][0:64, sl], pqk[0:64, :])
                    nc.vector.tensor_copy(dst[h0 + 1][64:128, sl], pqk[64:128, :])

        # ---------------- Phase 2: causal attention ----------------
        yT_pool = ctx.enter_context(tc.tile_pool(name="yT_pool", bufs=1))
        yT = [yT_pool.tile([128, T], BF16, name=f"yT{i}") for i in range(4)]

        with tc.tile_pool(name="att_pool", bufs=6) as att_pool, \
             tc.tile_pool(name="nrm_pool", bufs=2) as nrm_pool, \
             tc.tile_pool(name="ps_s_pool", bufs=2, space="PSUM") as ps_s_pool, \
             tc.tile_pool(name="ps_y_pool", bufs=3, space="PSUM") as ps_y_pool:

            def normalize(ps_y, h, cch):
                # yT[d, q] /= sums[q] (sums live in the ones-row 64 of ps_y).
                sums_sb = nrm_pool.tile([1, 512], F32R, tag="sums")
                nc.vector.tensor_copy(sums_sb[:], ps_y[64:65, :])
                # Broadcast sums across 64 partitions on the PE.
                ps_b = ps_y_pool.tile([64, 512], F32, name="ps_b", tag="ps_b",
                                      bufs=1)
                nc.tensor.matmul(ps_b[:], ones_col[:], sums_sb[:],
                                 start=True, stop=True)
                inv_sb = nrm_pool.tile([64, 512], F32, tag="inv")
                nc.vector.reciprocal_approx_fast(inv_sb[:], ps_b[:])
                ct, sl = h // 2, slice(cch * 512, (cch + 1) * 512)
                if h % 2 == 0:
                    nc.vector.tensor_tensor(
                        out=yT[ct][0:64, sl], in0=ps_y[0:64, :],
                        in1=inv_sb[:], op=MUL)
                else:
                    ystg = nrm_pool.tile([64, 512], BF16, tag="ystg")
                    nc.vector.tensor_tensor(
                        out=ystg[:], in0=ps_y[0:64, :],
                        in1=inv_sb[:], op=MUL)
                    nc.sync.dma_start(yT[ct][64:128, sl], ystg[:])

            def attn_steps(h, c2, j, ps_y0, ps_y1):
                """Emit QK -> exp -> mask -> AV for one (head, super-chunk, j)."""
                jmax = 8 * c2 + 7
                q0 = c2 * 1024
                dead = (j - 8 * c2) * 128 if j >= 8 * c2 else 0
                ps_s = ps_s_pool.tile([128, 1024], F32, name="ps_s", tag="ps_s")
                if dead < 512:
                    nc.tensor.matmul(
                        ps_s[:, dead:512],
                        kTt[h][:, j * 128:(j + 1) * 128],
                        qTt[h][:, q0 + dead:q0 + 512],
                        start=True, stop=True)
                lo_s = max(512, dead)
                nc.tensor.matmul(
                    ps_s[:, lo_s:1024],
                    kTt[h][:, j * 128:(j + 1) * 128],
                    qTt[h][:, q0 + lo_s:q0 + 1024],
                    start=True, stop=True)
                att = att_pool.tile([128, 1024], BF16, tag="att")
                nc.scalar.activation(
                    att[:, dead:1024], ps_s[:, dead:1024], EXP, scale=0.125)
                if j >= 8 * c2:
                    nc.gpsimd.tensor_tensor(
                        out=att[:, dead:dead + 128],
                        in0=att[:, dead:dead + 128],
                        in1=utri[:, :], op=MUL)
                # AV into the two 512-wide halves.
                if j <= 8 * c2 + 3:
                    nc.tensor.matmul(
                        ps_y0[:, dead:512], v_aug[:, j, h, :], att[:, dead:512],
                        start=(j == 0), stop=(j == 8 * c2 + 3))
                lo1 = max(512, dead)
                nc.tensor.matmul(
                    ps_y1[:, lo1 - 512:512], v_aug[:, j, h, :], att[:, lo1:1024],
                    start=(j == 0), stop=(j == jmax))

            for h in range(HG):
                for c2 in range(2):
                    ps_y0 = ps_y_pool.tile([65, 512], F32, name="ps_y0", tag="ps_y")
                    ps_y1 = ps_y_pool.tile([65, 512], F32, name="ps_y1", tag="ps_y")
                    for j in range(8 * c2 + 8):
                        attn_steps(h, c2, j, ps_y0, ps_y1)
                    normalize(ps_y0, h, 2 * c2)
                    normalize(ps_y1, h, 2 * c2 + 1)

        # ---------------- Phase 3: output projection ----------------
        with tc.tile_pool(name="out_pool", bufs=2) as out_pool, \
             tc.tile_pool(name="p3ps", bufs=2, space="PSUM") as p3ps:
            for tb in range(TB):
                o_sb = out_pool.tile([128, C], F32, tag="o_sb")
                for ch in range(2):
                    pp = p3ps.tile([128, 512], F32, tag="pp")
                    for ct in range(4):
                        nc.tensor.matmul(
                            pp[:],
                            yT[ct][:, tb * 128:(tb + 1) * 128],
                            wp_sb[ct][:, ch * 512:(ch + 1) * 512],
                            start=(ct == 0), stop=(ct == 3))
                    nc.vector.tensor_copy(o_sb[:, ch * 512:(ch + 1) * 512], pp[:])
                nc.sync.dma_start(out_d.ap()[tb * 128:(tb + 1) * 128, :], o_sb[:])

    nc.compile()
    return nc


def _get_nc():
    if "nc" not in _NC_CACHE:
        _register_ntff_hook()
        _NC_CACHE["nc"] = _build()
    return _NC_CACHE["nc"]


def kernel(x, w_attn, b_attn, w_proj, b_proj, _run_kwargs=None):
    import ml_dtypes
    from concourse.bass_utils import run_bass_kernel_spmd

    bf16 = ml_dtypes.bfloat16
    x = np.asarray(x, dtype=np.float32)
    w_attn = np.asarray(w_attn, dtype=np.float32)
    b_attn = np.asarray(b_attn, dtype=np.float32)
    w_proj = np.asarray(w_proj, dtype=np.float32)
    b_proj = np.asarray(b_proj, dtype=np.float32)

    nc = _get_nc()
    in_maps = []
    for core in range(NCORES):
        b, g = divmod(core, 2)
        cols = slice(g * CG, (g + 1) * CG)
        in_maps.append({
            "xT": np.ascontiguousarray(x[b].T).astype(bf16),
            "wqk": np.concatenate(
                [w_attn[:, cols], w_attn[:, C + g * CG: C + (g + 1) * CG]],
                axis=1).astype(bf16),
            "wv": np.ascontiguousarray(
                w_attn[:, 2 * C + g * CG: 2 * C + (g + 1) * CG]).astype(bf16),
            "wp": np.ascontiguousarray(w_proj[g * CG:(g + 1) * CG, :]).astype(bf16),
            "bqk": np.concatenate(
                [b_attn[cols], b_attn[C + g * CG: C + (g + 1) * CG]]
            ).reshape(1, -1).astype(bf16),
            "bv": np.ascontiguousarray(
                b_attn[2 * C + g * CG: 2 * C + (g + 1) * CG]).reshape(1, -1).astype(bf16),
        })

    res = run_bass_kernel_spmd(nc, in_maps, core_ids=list(range(NCORES)),
                               **(_run_kwargs or {}))
    out = np.empty((B, T, C), dtype=np.float32)
    for b in range(B):
        out[b] = res.results[2 * b]["out"] + res.results[2 * b + 1]["out"] + b_proj
    if _run_kwargs:
        kernel.last_results = res
    return out


# revision 30
# speedup vs baseline: 1.0412x; 1.0412x over previous
"""Causal self-attention (B=4, T=2048, C=1024, H=16, D=64) on 8 TRN2 NeuronCores.

Sharding: 8 cores = 4 batches x 2 head-groups (8 heads each). Each core:
  - QKV projection for its (batch, head-group) column slice of w_attn,
    producing qT/kT in [d, t] layout (transposed dataflow) and v in [t, d].
  - Causal attention in scoresT layout (scores^T[k, q] comes straight out of
    the PE; softmax denominators via an appended ones-column on V; no PE
    transposes anywhere).
  - Row-sharded output projection -> per-core partial [T, C].
Host sums the two partials per batch and adds b_proj.

Matmul operands are bf16 (1 cycle/row on the PE; fp32r streams at 2) with all
accumulation in fp32 PSUM. b_attn is folded in on-device via K=1 bias
matmuls; b_proj is added on the host during the unshard reduction.
"""

import sys
import types

import numpy as np

B, T, C, H, D = 4, 2048, 1024, 16, 64
HG = 8            # heads per core
CG = HG * D       # 512 channels per group
NCORES = 8
TB = T // 128     # 16 t-blocks
QCH = T // 512    # 4 q-chunks of 512


def _register_ntff_hook():
    """Register the axon NTFF profile hook if the image's antenv lacks it."""
    try:
        import antenv
        if getattr(antenv, "axon_hooks", None) is not None:
            return
        from trn_agent_boot.trn_boot import _ntff_profile_via_ctypes
        hook = _ntff_profile_via_ctypes("/opt/axon/libaxon_pjrt.so")
        mod = types.ModuleType("antenv.axon_hooks")
        mod._hook = hook
        mod.get_axon_ntff_profile_hook = lambda: mod._hook
        mod.set_axon_ntff_profile_hook = lambda h: setattr(mod, "_hook", h)
        sys.modules["antenv.axon_hooks"] = mod
        antenv.axon_hooks = mod
    except Exception:
        pass


_NC_CACHE = {}


def _build():
    import concourse.bacc as bacc
    import concourse.mybir as mybir
    import concourse.tile as tile
    from concourse.masks import make_upper_triangular
    from contextlib import ExitStack

    F32 = mybir.dt.float32
    F32R = mybir.dt.float32r
    BF16 = mybir.dt.bfloat16
    MUL = mybir.AluOpType.mult
    EXP = mybir.ActivationFunctionType.Exp

    nc = bacc.Bacc(None, target_bir_lowering=False, debug=False)
    xT_d = nc.dram_tensor("xT", [C, T], BF16, kind="ExternalInput")
    wqk_d = nc.dram_tensor("wqk", [C, 2 * CG], BF16, kind="ExternalInput")
    wv_d = nc.dram_tensor("wv", [C, CG], BF16, kind="ExternalInput")
    wp_d = nc.dram_tensor("wp", [CG, C], BF16, kind="ExternalInput")
    bqk_d = nc.dram_tensor("bqk", [1, 2 * CG], BF16, kind="ExternalInput")
    bv_d = nc.dram_tensor("bv", [1, CG], BF16, kind="ExternalInput")
    out_d = nc.dram_tensor("out", [T, C], F32, kind="ExternalOutput")

    CT = C // 128  # 8 c-tiles of the contraction dim

    with tile.TileContext(nc) as tc, ExitStack() as ctx:
        pers = ctx.enter_context(tc.tile_pool(name="pers", bufs=1))

        # Persistent tensors.
        # Per-head qT/kT tiles in [d, t] layout. Head h's 64 d-rows live at
        # partitions (h%2)*64..(h%2)*64+64 (matching the projection PSUM
        # layout); the other 64 partitions are zeroed so QK matmuls contract
        # over a full K=128 (keeps the PE HAM clock warm; zeros add nothing).
        qTt = [pers.tile([128, T], BF16, name=f"qTt{h}") for h in range(HG)]
        kTt = [pers.tile([128, T], BF16, name=f"kTt{h}") for h in range(HG)]
        for h in range(HG):
            zs = slice(64, 128) if h % 2 == 0 else slice(0, 64)
            nc.gpsimd.memset(qTt[h][zs, :], 0.0)
            nc.gpsimd.memset(kTt[h][zs, :], 0.0)
        # v_aug[p, j, h, 0:64] = v[t=j*128+p, h*64+d]; [..., 64] = 1.0
        v_aug = pers.tile([128, TB, HG, 65], BF16, name="v_aug")
        utri = pers.tile([128, 128], BF16, name="utri")
        ones_col = pers.tile([1, 64], F32R, name="ones_col")
        ones_q = pers.tile([1, 512], BF16, name="ones_q")
        bqk_sb = pers.tile([1, 2 * CG], BF16, name="bqk_sb")
        bv_sb = pers.tile([1, CG], BF16, name="bv_sb")

        # f32r/bf16 constants staged via f32 memset + rounding copies.
        stage = pers.tile([128, 512], F32, name="stage")
        make_upper_triangular(nc, utri[:, :], val=1.0, diag=True)
        nc.vector.memset(stage[:], 1.0)
        nc.vector.tensor_copy(ones_col[:], stage[0:1, 0:64])
        nc.vector.tensor_copy(ones_q[:], stage[0:1, :])
        nc.vector.tensor_copy(
            v_aug[:, :, :, 64:65],
            stage[:, 0:128].rearrange("p (j h) -> p j h", j=TB))
        nc.sync.dma_start(bqk_sb[:], bqk_d.ap()[:])
        nc.sync.dma_start(bv_sb[:], bv_d.ap()[:])

        # Weights for the output projection, DMA'd early (space is free).
        wp_pool = ctx.enter_context(tc.tile_pool(name="wp_pool", bufs=1))
        wp_sb = [wp_pool.tile([128, C], BF16, name=f"wp{i}") for i in range(4)]
        for i in range(4):
            nc.sync.dma_start(wp_sb[i][:], wp_d.ap()[i * 128:(i + 1) * 128, :])

        # ---------------- Phase 1: QKV projection ----------------
        with tc.tile_pool(name="wqk_pool", bufs=1) as wqk_pool, \
             tc.tile_pool(name="wv_pool", bufs=1) as wv_pool, \
             tc.tile_pool(name="xq_pool", bufs=2) as xq_pool, \
             tc.tile_pool(name="p1ps", bufs=2, space="PSUM") as p1ps:
            wqk_sb = [wqk_pool.tile([128, 2 * CG], BF16, name=f"wqk{c}")
                      for c in range(CT)]
            wv_sb = [wv_pool.tile([128, CG], BF16, name=f"wv{c}")
                     for c in range(CT)]
            for c in range(CT):
                nc.sync.dma_start(wqk_sb[c][:], wqk_d.ap()[c * 128:(c + 1) * 128, :])
                nc.sync.dma_start(wv_sb[c][:], wv_d.ap()[c * 128:(c + 1) * 128, :])

            for q in range(QCH):  # t-quarters of 512
                xq = []
                for c in range(CT):
                    xt = xq_pool.tile([128, 512], BF16, name=f"xq{c}", tag=f"xq{c}")
                    nc.sync.dma_start(
                        xt[:], xT_d.ap()[c * 128:(c + 1) * 128, q * 512:(q + 1) * 512])
                    xq.append(xt)

                # V projection: v[t, c'] for the 4 t-blocks of this quarter.
                for tb in range(4):
                    pv = p1ps.tile([128, CG], F32, tag="pv")
                    for c in range(CT):
                        nc.tensor.matmul(
                            pv[:], xq[c][:, tb * 128:(tb + 1) * 128], wv_sb[c][:],
                            start=(c == 0), stop=False)
                    nc.tensor.matmul(
                        pv[:], ones_q[:, tb * 128:(tb + 1) * 128], bv_sb[:],
                        start=False, stop=True)
                    j = q * 4 + tb
                    nc.vector.tensor_copy(
                        v_aug[:, j, :, 0:64],
                        pv[:].rearrange("p (h d) -> p h d", h=HG))

                # Q/K projection (transposed). M-block m covers heads
                # (2(m%4), 2(m%4)+1) of Q (m<4) or K (m>=4).
                for m in range(8):
                    pqk = p1ps.tile([128, 512], F32, tag="pqk")
                    for c in range(CT):
                        nc.tensor.matmul(
                            pqk[:], wqk_sb[c][:, m * 128:(m + 1) * 128], xq[c][:],
                            start=(c == 0), stop=False)
                    nc.tensor.matmul(
                        pqk[:], bqk_sb[:, m * 128:(m + 1) * 128], ones_q[:],
                        start=False, stop=True)
                    dst = qTt if m < 4 else kTt
                    h0 = 2 * (m % 4)
                    sl = slice(q * 512, (q + 1) * 512)
                    nc.vector.tensor_copy(dst[h0][0:64, sl], pqk[0:64, :])
                    nc.vector.tensor_copy(dst[h0 + 1][64:128, sl], pqk[64:128, :])

        # ---------------- Phase 2: causal attention ----------------
        yT_pool = ctx.enter_context(tc.tile_pool(name="yT_pool", bufs=1))
        yT = [yT_pool.tile([128, T], BF16, name=f"yT{i}") for i in range(4)]

        with tc.tile_pool(name="att_pool", bufs=6) as att_pool, \
             tc.tile_pool(name="nrm_pool", bufs=2) as nrm_pool, \
             tc.tile_pool(name="ps_s_pool", bufs=2, space="PSUM") as ps_s_pool, \
             tc.tile_pool(name="ps_y_pool", bufs=3, space="PSUM") as ps_y_pool:

            def normalize(ps_y, h, cch):
                # yT[d, q] /= sums[q] (sums live in the ones-row 64 of ps_y).
                sums_sb = nrm_pool.tile([1, 512], F32R, tag="sums")
                nc.vector.tensor_copy(sums_sb[:], ps_y[64:65, :])
                # Broadcast sums across 64 partitions on the PE.
                ps_b = ps_y_pool.tile([64, 512], F32, name="ps_b", tag="ps_b",
                                      bufs=1)
                nc.tensor.matmul(ps_b[:], ones_col[:], sums_sb[:],
                                 start=True, stop=True)
                inv_sb = nrm_pool.tile([64, 512], F32, tag="inv")
                nc.vector.reciprocal_approx_fast(inv_sb[:], ps_b[:])
                ct, sl = h // 2, slice(cch * 512, (cch + 1) * 512)
                if h % 2 == 0:
                    nc.vector.tensor_tensor(
                        out=yT[ct][0:64, sl], in0=ps_y[0:64, :],
                        in1=inv_sb[:], op=MUL)
                else:
                    ystg = nrm_pool.tile([64, 512], BF16, tag="ystg")
                    nc.vector.tensor_tensor(
                        out=ystg[:], in0=ps_y[0:64, :],
                        in1=inv_sb[:], op=MUL)
                    nc.sync.dma_start(yT[ct][64:128, sl], ystg[:])

            def attn_steps(h, c2, j, ps_y0, ps_y1):
                """Emit QK -> exp -> mask -> AV for one (head, super-chunk, j)."""
                jmax = 8 * c2 + 7
                q0 = c2 * 1024
                dead = (j - 8 * c2) * 128 if j >= 8 * c2 else 0
                ps_s = ps_s_pool.tile([128, 1024], F32, name="ps_s", tag="ps_s")
                if dead < 512:
                    nc.tensor.matmul(
                        ps_s[:, dead:512],
                        kTt[h][:, j * 128:(j + 1) * 128],
                        qTt[h][:, q0 + dead:q0 + 512],
                        start=True, stop=True)
                lo_s = max(512, dead)
                nc.tensor.matmul(
                    ps_s[:, lo_s:1024],
                    kTt[h][:, j * 128:(j + 1) * 128],
                    qTt[h][:, q0 + lo_s:q0 + 1024],
                    start=True, stop=True)
                att = att_pool.tile([128, 1024], BF16, tag="att")
                nc.scalar.activation(
                    att[:, dead:1024], ps_s[:, dead:1024], EXP, scale=0.125)
                if j >= 8 * c2:
                    nc.vector.tensor_tensor(
                        out=att[:, dead:dead + 128],
                        in0=att[:, dead:dead + 128],
                        in1=utri[:, :], op=MUL)
                # AV into the two 512-wide halves.
                if j <= 8 * c2 + 3:
                    nc.tensor.matmul(
                        ps_y0[:, dead:512], v_aug[:, j, h, :], att[:, dead:512],
                        start=(j == 0), stop=(j == 8 * c2 + 3))
                lo1 = max(512, dead)
                nc.tensor.matmul(
                    ps_y1[:, lo1 - 512:512], v_aug[:, j, h, :], att[:, lo1:1024],
                    start=(j == 0), stop=(j == jmax))

            for h in range(HG):
                for c2 in range(2):
                    ps_y0 = ps_y_pool.tile([65, 512], F32, name="ps_y0", tag="ps_y")
                    ps_y1 = ps_y_pool.tile([65, 512], F32, name="ps_y1", tag="ps_y")
                    for j in range(8 * c2 + 8):
                        attn_steps(h, c2, j, ps_y0, ps_y1)
                    normalize(ps_y0, h, 2 * c2)
                    normalize(ps_y1, h, 2 * c2 + 1)

        # ---------------- Phase 3: output projection ----------------
        with tc.tile_pool(name="out_pool", bufs=2) as out_pool, \
             tc.tile_pool(name="p3ps", bufs=2, space="PSUM") as p3ps:
            for tb in range(TB):
                o_sb = out_pool.tile([128, C], F32, tag="o_sb")
                for ch in range(2):
                    pp = p3ps.tile([128, 512], F32, tag="pp")
                    for ct in range(4):
                        nc.tensor.matmul(
                            pp[:],
                            yT[ct][:, tb * 128:(tb + 1) * 128],
                            wp_sb[ct][:, ch * 512:(ch + 1) * 512],
                            start=(ct == 0), stop=(ct == 3))
                    nc.vector.tensor_copy(o_sb[:, ch * 512:(ch + 1) * 512], pp[:])
                nc.sync.dma_start(out_d.ap()[tb * 128:(tb + 1) * 128, :], o_sb[:])

    nc.compile()
    return nc


def _get_nc():
    if "nc" not in _NC_CACHE:
        _register_ntff_hook()
        _NC_CACHE["nc"] = _build()
    return _NC_CACHE["nc"]


def kernel(x, w_attn, b_attn, w_proj, b_proj, _run_kwargs=None):
    import ml_dtypes
    from concourse.bass_utils import run_bass_kernel_spmd

    bf16 = ml_dtypes.bfloat16
    x = np.asarray(x, dtype=np.float32)
    w_attn = np.asarray(w_attn, dtype=np.float32)
    b_attn = np.asarray(b_attn, dtype=np.float32)
    w_proj = np.asarray(w_proj, dtype=np.float32)
    b_proj = np.asarray(b_proj, dtype=np.float32)

    nc = _get_nc()
    in_maps = []
    for core in range(NCORES):
        b, g = divmod(core, 2)
        cols = slice(g * CG, (g + 1) * CG)
        in_maps.append({
            "xT": np.ascontiguousarray(x[b].T).astype(bf16),
            "wqk": np.concatenate(
                [w_attn[:, cols], w_attn[:, C + g * CG: C + (g + 1) * CG]],
                axis=1).astype(bf16),
            "wv": np.ascontiguousarray(
                w_attn[:, 2 * C + g * CG: 2 * C + (g + 1) * CG]).astype(bf16),
            "wp": np.ascontiguousarray(w_proj[g * CG:(g + 1) * CG, :]).astype(bf16),
            "bqk": np.concatenate(
                [b_attn[cols], b_attn[C + g * CG: C + (g + 1) * CG]]
            ).reshape(1, -1).astype(bf16),
            "bv": np.ascontiguousarray(
                b_attn[2 * C + g * CG: 2 * C + (g + 1) * CG]).reshape(1, -1).astype(bf16),
        })

    res = run_bass_kernel_spmd(nc, in_maps, core_ids=list(range(NCORES)),
                               **(_run_kwargs or {}))
    out = np.empty((B, T, C), dtype=np.float32)
    for b in range(B):
        out[b] = res.results[2 * b]["out"] + res.results[2 * b + 1]["out"] + b_proj
    if _run_kwargs:
        kernel.last_results = res
    return out
